# revision 31
# baseline (speedup 1.0000x reference)
"""Autoregressive GRU decoder on 8 TRN2 NeuronCores (data-parallel over batch).

Math (per step, reference semantics):
    px   = x * w_proj + b_proj                      # x is scalar per row
    gi   = px @ W_ih.T + b_ih                       # rank-1 in x:
         = x * u + c,   u = W_ih @ w_proj,  c = W_ih @ b_proj + b_ih
    gh   = h @ W_hh.T + b_hh
    r    = sigmoid(gi_r + gh_r);  z = sigmoid(gi_z + gh_z)
    n    = tanh(gi_n + r * gh_n')                   # gh_n' = gh_n (incl b_hh_n)
    h    = n + z * (h - n)
    pred = h @ w_out + b_out;  x_next = pred

Device layout is feature-major (hidden dim on partitions, batch on the free
dim) so the recurrent matmul needs no transposes; the host pre-transposes
encoder_out and W_hh and precomputes u/c (tiny: ~0.4 MFLOP of weight prep).

Implementation notes (per core: 2048 batch rows, 4 column-chunks of 512):
- bf16 datapath, f32 PSUM accumulation (rel err vs f32 reference ~7.5e-3).
- PE: per chunk 8x K=128 main matmuls into r/z PSUM, 4x into n PSUM, plus the
  rank-1 [u*x; d*1] folds as 8 K=1 matmuls packed into distinct 32-row groups
  via tile_position (concurrent on silicon), and a K=256 matvec for pred.
- ACT: fused [128,1024] sigmoids (r, z) and tanh; per-partition bias APs carry
  c_n; scalar_tensor_tensor on DVE carries b_hh_n and the gi_n add.
- Pool(GpSimd): partition_broadcast of the step input x, and the h-update
  chain (h-n, z*(h-n), h_new) to offload DVE.
- PSUM: 8 banks = g_r(2) + g_z(2) + g_n(2 dbl-buffered) + pred(2 slots);
  r/z split into separate tensors so the next chunk's matmuls only wait on
  the sigmoid that drained first.
- Emission is software-pipelined: each chunk's pred/x-write is emitted one
  chunk late so the PE stream never waits on the gate chain (sim: zero PE
  gaps >200ns in steady state; 95% PE occupancy at serial pricing).
"""

import sys

import numpy as np

if "/opt/trn_rl_repo" not in sys.path:
    sys.path.insert(0, "/opt/trn_rl_repo")

N = 16384
H = 256
H3 = 3 * H
T = 24
NCORES = 8
R = N // NCORES  # 2048 rows per core
CH = 512  # batch-column chunk (one PSUM bank at f32)
NCH = R // CH

_CACHE: dict = {}


def _build():
    import concourse.bass as bass
    import concourse.bacc as bacc
    import concourse.mybir as mybir
    from concourse.tile import TileContext

    f32 = mybir.dt.float32
    bf16 = mybir.dt.bfloat16
    AF = mybir.ActivationFunctionType
    OP = mybir.AluOpType

    nc = bacc.Bacc()

    h0T_d = nc.declare_dram_parameter("h0T", [H, R], bf16, isOutput=False)
    whhT_d = nc.declare_dram_parameter("whhT", [H, H3], bf16, isOutput=False)
    s3xu_d = nc.declare_dram_parameter("s3xu", [128, H3], bf16, isOutput=False)
    s3xd_d = nc.declare_dram_parameter("s3xd", [128, H3], bf16, isOutput=False)
    uc_d = nc.declare_dram_parameter("uc", [2, 128, 3], f32, isOutput=False)
    wo_d = nc.declare_dram_parameter("wo", [2, 128, 1], bf16, isOutput=False)
    bo_d = nc.declare_dram_parameter("bo", [1, 1], f32, isOutput=False)
    out_d = nc.declare_dram_parameter("out", [T, R], bf16, isOutput=True)

    from contextlib import ExitStack

    with TileContext(nc) as tc, ExitStack() as stack:
        persist = stack.enter_context(tc.tile_pool(name="persist", bufs=1))

        def mk(shape, name, dt=None):
            return persist.tile(shape, dt if dt is not None else bf16,
                                name=name, tag=name)

        # ---- persistent SBUF state ----
        hT0 = mk([128, R], "hT0")   # hidden rows 0:128, batch on free dim
        hT1 = mk([128, R], "hT1")   # hidden rows 128:256
        wT0 = mk([128, H3], "wT0")  # W_hh.T rows 0:128
        wT1 = mk([128, H3], "wT1")  # W_hh.T rows 128:256
        s3xu = mk([128, H3], "s3xu")  # u rows at partitions 32m (K3a stationary)
        s3xd = mk([128, H3], "s3xd")  # d rows at partitions 32m (K3b stationary)
        uc0 = mk([128, 3], "uc0", f32)  # cols: u_n, c_n, b_hh_n (hidden 0:128)
        uc1 = mk([128, 3], "uc1", f32)
        wo0 = mk([128, 1], "wo0")
        wo1 = mk([128, 1], "wo1")
        bo = mk([1, 1], "bo", f32)
        oneb = mk([128, R], "oneb")  # static ones (K3b moving rows)
        # double-buffered x rows: row0 = current x (pred), row1 = ones
        xoA = mk([2, R], "xoA")
        xoB = mk([2, R], "xoB")
        xb = mk([128, R], "xb")     # x broadcast across partitions
        gi0 = mk([128, R], "gi0")   # x*u_n + c_n (hidden 0:128)
        gi1 = mk([128, R], "gi1")

        nc.sync.dma_start(out=hT0[:], in_=h0T_d[0:128, :])
        nc.sync.dma_start(out=hT1[:], in_=h0T_d[128:256, :])
        nc.sync.dma_start(out=wT0[:], in_=whhT_d[0:128, :])
        nc.sync.dma_start(out=wT1[:], in_=whhT_d[128:256, :])
        nc.sync.dma_start(out=s3xu[:], in_=s3xu_d[:])
        nc.sync.dma_start(out=s3xd[:], in_=s3xd_d[:])
        nc.sync.dma_start(out=uc0[:], in_=uc_d[0])
        nc.sync.dma_start(out=uc1[:], in_=uc_d[1])
        nc.sync.dma_start(out=wo0[:], in_=wo_d[0])
        nc.sync.dma_start(out=wo1[:], in_=wo_d[1])
        nc.sync.dma_start(out=bo[:], in_=bo_d[:])
        nc.vector.memset(oneb[:], 1.0)
        nc.vector.memset(xoA[0:2, :], 1.0)   # row1 stays 1.0 forever
        nc.vector.memset(xoA[0:1, :], 0.0)   # row0 = x_0 = 0
        nc.vector.memset(xoB[0:2, :], 1.0)

        with (
            tc.tile_pool(name="gr", bufs=1, space="PSUM") as grpool,
            tc.tile_pool(name="gz", bufs=1, space="PSUM") as gzpool,
            tc.tile_pool(name="gn", bufs=2, space="PSUM") as gnpool,
            tc.tile_pool(name="sm", bufs=2, space="PSUM") as smpool,
            tc.tile_pool(name="rz", bufs=3) as rzpool,
            tc.tile_pool(name="wk", bufs=6) as wkpool,
        ):
            def emit_tail(pend):
                """pred + x/out write for a finished chunk (pipelined)."""
                tt, cc, xo_t = pend
                slc = slice(cc * CH, (cc + 1) * CH)
                pp = smpool.tile([1, CH], f32, tag="sm", name="pp")
                nc.tensor.matmul(pp[:], wo0[:], hT0[:, slc],
                                 start=True, stop=False)
                nc.tensor.matmul(pp[:], wo1[:], hT1[:, slc],
                                 start=False, stop=True)
                nc.vector.tensor_scalar_add(xo_t[0:1, slc], pp[:], bo[0:1, 0:1])
                if cc == NCH - 1:
                    nc.sync.dma_start(out=out_d[tt : tt + 1, :], in_=xo_t[0:1, :])

            pending = None
            for t in range(T):
                xin = xoA if t % 2 == 0 else xoB   # x_t lives here
                xout = xoB if t % 2 == 0 else xoA  # pred_t written here
                for c in range(NCH):
                    sl = slice(c * CH, (c + 1) * CH)
                    # x broadcast + gi_n for this chunk
                    nc.gpsimd.partition_broadcast(xb[:, sl], xin[0:1, sl])
                    nc.vector.tensor_scalar(gi0[:, sl], xb[:, sl],
                                            uc0[:, 0:1], uc0[:, 1:2],
                                            OP.mult, OP.add)
                    nc.vector.tensor_scalar(gi1[:, sl], xb[:, sl],
                                            uc1[:, 0:1], uc1[:, 1:2],
                                            OP.mult, OP.add)
                    gr = grpool.tile([128, 2 * CH], f32, tag="gr", name="gr")
                    gz = gzpool.tile([128, 2 * CH], f32, tag="gz", name="gz")
                    rz = rzpool.tile([128, 4 * CH], bf16, tag="rz", name="rz")
                    def gsl(m):
                        return (gr if m < 2 else gz)[
                            :, (m % 2) * CH : (m % 2 + 1) * CH]

                    for m in range(4):
                        ms = slice(m * 128, (m + 1) * 128)
                        nc.tensor.matmul(gsl(m), wT0[:, ms], hT0[:, sl],
                                         start=True, stop=False)
                        nc.tensor.matmul(gsl(m), wT1[:, ms], hT1[:, sl],
                                         start=False, stop=False)
                    # rank-1 x-fold + bias: K=1 matmuls packed into distinct
                    # 32-row groups of the PE array -> run concurrently
                    for m in range(4):
                        nc.tensor.matmul(gsl(m),
                                         s3xu[32 * m : 32 * m + 1,
                                              m * 128 : (m + 1) * 128],
                                         xb[32 * m : 32 * m + 1, sl],
                                         start=False, stop=False,
                                         tile_position=(32 * m, 0))
                    for m in range(4):
                        nc.tensor.matmul(gsl(m),
                                         s3xd[32 * m : 32 * m + 1,
                                              m * 128 : (m + 1) * 128],
                                         oneb[32 * m : 32 * m + 1, sl],
                                         start=False, stop=True,
                                         tile_position=(32 * m, 0))
                    nc.scalar.activation(rz[:, 0 : 2 * CH], gr[:], AF.Sigmoid)
                    nc.scalar.activation(rz[:, 2 * CH : 4 * CH],
                                         gz[:], AF.Sigmoid)
                    gns = []
                    for tl in range(2):
                        gn = gnpool.tile([128, CH], f32, tag="gn", name="gn")
                        ms = slice((4 + tl) * 128, (5 + tl) * 128)
                        nc.tensor.matmul(gn[:], wT0[:, ms], hT0[:, sl],
                                         start=True, stop=False)
                        nc.tensor.matmul(gn[:], wT1[:, ms], hT1[:, sl],
                                         start=False, stop=True)
                        gns.append(gn)
                    # previous chunk's pred now that PE has fresh work queued
                    if pending is not None:
                        emit_tail(pending)
                    ta2 = wkpool.tile([128, 2 * CH], bf16, tag="ta2", name="ta2")
                    nn2 = wkpool.tile([128, 2 * CH], bf16, tag="nn2", name="nn2")
                    for tl, (hT, uc, gi) in enumerate(
                            ((hT0, uc0, gi0), (hT1, uc1, gi1))):
                        gn = gns[tl]
                        rr = rz[:, tl * CH : (tl + 1) * CH]
                        # rh = (gh_n + b_hh_n) * r
                        rh = wkpool.tile([128, CH], bf16, tag="rh", name="rh")
                        nc.vector.scalar_tensor_tensor(
                            rh[:], gn[:], uc[:, 2:3], rr, OP.add, OP.mult)
                        # ta = gi_n + rh
                        nc.vector.tensor_tensor(
                            ta2[:, tl * CH : (tl + 1) * CH], gi[:, sl], rh[:],
                            OP.add)
                    # n = tanh(ta) for both hidden tiles in one LUT pass
                    nc.scalar.activation(nn2[:], ta2[:], AF.Tanh)
                    for tl, hT in enumerate((hT0, hT1)):
                        nn = nn2[:, tl * CH : (tl + 1) * CH]
                        zz = rz[:, (2 + tl) * CH : (3 + tl) * CH]
                        hmn = wkpool.tile([128, CH], bf16, tag="hmn", name="hmn")
                        nc.gpsimd.tensor_tensor(hmn[:], hT[:, sl], nn, OP.subtract)
                        zh = wkpool.tile([128, CH], bf16, tag="zh", name="zh")
                        nc.gpsimd.tensor_tensor(zh[:], zz, hmn[:], OP.mult)
                        nc.gpsimd.tensor_tensor(hT[:, sl], nn, zh[:], OP.add)
                    pending = (t, c, xout)
            emit_tail(pending)

    nc.compile()
    return nc


def _prep_maps(encoder_out, w_proj, b_proj, W_ih, b_ih, W_hh, b_hh, w_out, b_out):
    f = np.float32
    u = (W_ih @ w_proj).astype(f)                    # [768]
    cvec = (W_ih @ b_proj + b_ih).astype(f)          # [768]
    s3 = np.zeros((2, H3), f)
    s3[0, : 2 * H] = u[: 2 * H]
    s3[1, : 2 * H] = cvec[: 2 * H] + b_hh[: 2 * H]
    s3[1, 2 * H :] = b_hh[2 * H :]
    s3xu = np.zeros((128, H3), f)
    s3xd = np.zeros((128, H3), f)
    for m in range(4):
        s3xu[32 * m, :] = s3[0]
        s3xd[32 * m, :] = s3[1]
    uc = np.zeros((2, 128, 3), f)
    uc[0, :, 0] = u[2 * H : 2 * H + 128]
    uc[0, :, 1] = cvec[2 * H : 2 * H + 128]
    uc[0, :, 2] = b_hh[2 * H : 2 * H + 128]
    uc[1, :, 0] = u[2 * H + 128 :]
    uc[1, :, 1] = cvec[2 * H + 128 :]
    uc[1, :, 2] = b_hh[2 * H + 128 :]
    import ml_dtypes

    bf = ml_dtypes.bfloat16
    wo = np.ascontiguousarray(w_out.astype(f).reshape(2, 128, 1)).astype(bf)
    bo = b_out.astype(f).reshape(1, 1)
    whhT = np.ascontiguousarray(W_hh.astype(f).T).astype(bf)  # [256, 768]
    s3xu = s3xu.astype(bf)
    s3xd = s3xd.astype(bf)
    maps = []
    for i in range(NCORES):
        h0T = np.ascontiguousarray(
            encoder_out[i * R : (i + 1) * R].astype(f).T
        ).astype(bf)
        maps.append(
            dict(h0T=h0T, whhT=whhT, s3xu=s3xu, s3xd=s3xd, uc=uc, wo=wo, bo=bo)
        )
    return maps


def _run(inputs, trace=False, **kw):
    import time

    from concourse.bass_utils import run_bass_kernel_spmd

    if "nc" not in _CACHE:
        _CACHE["nc"] = _build()
    nc = _CACHE["nc"]
    in_maps = _prep_maps(**inputs)
    res = None
    for attempt, pause in enumerate((0, 30, 120)):
        if pause:
            time.sleep(pause)  # transient NRT/axon device errors self-recover
        try:
            res = run_bass_kernel_spmd(nc, in_maps, core_ids=list(range(NCORES)),
                                       trace=trace, **kw)
            break
        except Exception:
            if attempt == 2:
                raise
    full = np.empty((N, T), np.float32)
    for i in range(NCORES):
        o = np.asarray(res.results[i]["out"]).astype(np.float32)
        full[i * R : (i + 1) * R] = o.T
    return full, res


def kernel(**inputs):
    inputs = {k: np.asarray(v) for k, v in inputs.items()}
    full, _ = _run(inputs)
    return full


# revision 32
# speedup vs baseline: 1.1170x; 1.1170x over previous
"""Autoregressive GRU decoder on 8 TRN2 NeuronCores (data-parallel over batch).

Math (per step, reference semantics):
    px   = x * w_proj + b_proj                      # x is scalar per row
    gi   = px @ W_ih.T + b_ih                       # rank-1 in x:
         = x * u + c,   u = W_ih @ w_proj,  c = W_ih @ b_proj + b_ih
    gh   = h @ W_hh.T + b_hh
    r    = sigmoid(gi_r + gh_r);  z = sigmoid(gi_z + gh_z)
    n    = tanh(gi_n + r * gh_n')                   # gh_n' = gh_n (incl b_hh_n)
    h    = n + z * (h - n)
    pred = h @ w_out + b_out;  x_next = pred

Device layout is feature-major (hidden dim on partitions, batch on the free
dim) so the recurrent matmul needs no transposes; the host pre-transposes
encoder_out and W_hh and precomputes u/c (tiny: ~0.4 MFLOP of weight prep).

Implementation notes (per core: 2048 batch rows, 4 column-chunks of 512):
- bf16 datapath, f32 PSUM accumulation (rel err vs f32 reference ~7.5e-3).
- PE: per chunk 8x K=128 main matmuls into r/z PSUM, 4x into n PSUM, plus the
  rank-1 [u*x; d*1] folds as 8 K=1 matmuls packed into distinct 32-row groups
  via tile_position (concurrent on silicon), and a K=256 matvec for pred.
- ACT: fused [128,1024] sigmoids (r, z) and tanh; per-partition bias APs carry
  c_n; scalar_tensor_tensor on DVE carries b_hh_n and the gi_n add.
- Pool(GpSimd): partition_broadcast of the step input x, and the h-update
  chain (h-n, z*(h-n), h_new) to offload DVE.
- PSUM: 8 banks = g_r(2) + g_z(2) + g_n(2 dbl-buffered) + pred(2 slots);
  r/z split into separate tensors so the next chunk's matmuls only wait on
  the sigmoid that drained first.
- Emission is software-pipelined: each chunk's pred/x-write is emitted one
  chunk late so the PE stream never waits on the gate chain (sim: zero PE
  gaps >200ns in steady state; 95% PE occupancy at serial pricing).
"""

import sys

import numpy as np

if "/opt/trn_rl_repo" not in sys.path:
    sys.path.insert(0, "/opt/trn_rl_repo")

N = 16384
H = 256
H3 = 3 * H
T = 24
NCORES = 8
R = N // NCORES  # 2048 rows per core
CH = 512  # batch-column chunk (one PSUM bank at f32)
NCH = R // CH

_CACHE: dict = {}


def _build():
    import concourse.bass as bass
    import concourse.bacc as bacc
    import concourse.mybir as mybir
    from concourse.tile import TileContext

    f32 = mybir.dt.float32
    bf16 = mybir.dt.bfloat16
    AF = mybir.ActivationFunctionType
    OP = mybir.AluOpType

    nc = bacc.Bacc()

    h0T_d = nc.declare_dram_parameter("h0T", [H, R], bf16, isOutput=False)
    whhT_d = nc.declare_dram_parameter("whhT", [H, H3], bf16, isOutput=False)
    s3x2_d = nc.declare_dram_parameter("s3x2", [128, H3], bf16, isOutput=False)
    uc_d = nc.declare_dram_parameter("uc", [2, 128, 3], f32, isOutput=False)
    wo_d = nc.declare_dram_parameter("wo", [2, 128, 1], bf16, isOutput=False)
    bo_d = nc.declare_dram_parameter("bo", [1, 1], f32, isOutput=False)
    out_d = nc.declare_dram_parameter("out", [T, R], bf16, isOutput=True)

    from contextlib import ExitStack

    with TileContext(nc) as tc, ExitStack() as stack:
        persist = stack.enter_context(tc.tile_pool(name="persist", bufs=1))

        def mk(shape, name, dt=None):
            return persist.tile(shape, dt if dt is not None else bf16,
                                name=name, tag=name)

        # ---- persistent SBUF state ----
        hT0 = mk([128, R], "hT0")   # hidden rows 0:128, batch on free dim
        hT1 = mk([128, R], "hT1")   # hidden rows 128:256
        wT0 = mk([128, H3], "wT0")  # W_hh.T rows 0:128
        wT1 = mk([128, H3], "wT1")  # W_hh.T rows 128:256
        s3x2 = mk([128, H3], "s3x2")  # [u;d] row pairs at partitions {32m,32m+1}
        uc0 = mk([128, 3], "uc0", f32)  # cols: u_n, c_n, b_hh_n (hidden 0:128)
        uc1 = mk([128, 3], "uc1", f32)
        wo0 = mk([128, 1], "wo0")
        wo1 = mk([128, 1], "wo1")
        bo = mk([1, 1], "bo", f32)
        xob = mk([128, R], "xob")   # rows 32m = x (DMA), rows 32m+1 = 1.0
        # double-buffered x rows: row0 = current x (pred), row1 = ones
        xoA = mk([2, R], "xoA")
        xoB = mk([2, R], "xoB")
        xb = mk([128, R], "xb")     # x broadcast across partitions
        gi0 = mk([128, R], "gi0")   # x*u_n + c_n (hidden 0:128)
        gi1 = mk([128, R], "gi1")

        nc.sync.dma_start(out=hT0[:], in_=h0T_d[0:128, :])
        nc.sync.dma_start(out=hT1[:], in_=h0T_d[128:256, :])
        nc.sync.dma_start(out=wT0[:], in_=whhT_d[0:128, :])
        nc.sync.dma_start(out=wT1[:], in_=whhT_d[128:256, :])
        nc.sync.dma_start(out=s3x2[:], in_=s3x2_d[:])
        nc.sync.dma_start(out=uc0[:], in_=uc_d[0])
        nc.sync.dma_start(out=uc1[:], in_=uc_d[1])
        nc.sync.dma_start(out=wo0[:], in_=wo_d[0])
        nc.sync.dma_start(out=wo1[:], in_=wo_d[1])
        nc.sync.dma_start(out=bo[:], in_=bo_d[:])
        nc.vector.memset(xob[:], 1.0)  # x rows overwritten per chunk
        nc.vector.memset(xoA[0:2, :], 1.0)   # row1 stays 1.0 forever
        nc.vector.memset(xoA[0:1, :], 0.0)   # row0 = x_0 = 0
        nc.vector.memset(xoB[0:2, :], 1.0)

        with (
            tc.tile_pool(name="gr", bufs=1, space="PSUM") as grpool,
            tc.tile_pool(name="gz", bufs=1, space="PSUM") as gzpool,
            tc.tile_pool(name="gn", bufs=2, space="PSUM") as gnpool,
            tc.tile_pool(name="sm", bufs=2, space="PSUM") as smpool,
            tc.tile_pool(name="rz", bufs=3) as rzpool,
            tc.tile_pool(name="wk", bufs=6) as wkpool,
        ):
            def emit_tail(pend):
                """pred + x/out write for a finished chunk (pipelined)."""
                tt, cc, xo_t = pend
                slc = slice(cc * CH, (cc + 1) * CH)
                pp = smpool.tile([1, CH], f32, tag="sm", name="pp")
                nc.tensor.matmul(pp[:], wo0[:], hT0[:, slc],
                                 start=True, stop=False)
                nc.tensor.matmul(pp[:], wo1[:], hT1[:, slc],
                                 start=False, stop=True)
                nc.vector.tensor_scalar_add(xo_t[0:1, slc], pp[:], bo[0:1, 0:1])
                if cc == NCH - 1:
                    nc.sync.dma_start(out=out_d[tt : tt + 1, :], in_=xo_t[0:1, :])

            pending = None
            for t in range(T):
                xin = xoA if t % 2 == 0 else xoB   # x_t lives here
                xout = xoB if t % 2 == 0 else xoA  # pred_t written here
                for c in range(NCH):
                    sl = slice(c * CH, (c + 1) * CH)
                    # x broadcast + gi_n for this chunk; also place x at
                    # partitions 32/64/96 for the packed rank-1 matmuls
                    nc.gpsimd.partition_broadcast(xb[:, sl], xin[0:1, sl])
                    for q in (1, 2, 3):
                        nc.sync.dma_start(out=xob[32 * q : 32 * q + 1, sl],
                                          in_=xin[0:1, sl])
                    nc.vector.tensor_scalar(gi0[:, sl], xb[:, sl],
                                            uc0[:, 0:1], uc0[:, 1:2],
                                            OP.mult, OP.add)
                    nc.vector.tensor_scalar(gi1[:, sl], xb[:, sl],
                                            uc1[:, 0:1], uc1[:, 1:2],
                                            OP.mult, OP.add)
                    gr = grpool.tile([128, 2 * CH], f32, tag="gr", name="gr")
                    gz = gzpool.tile([128, 2 * CH], f32, tag="gz", name="gz")
                    rz = rzpool.tile([128, 4 * CH], bf16, tag="rz", name="rz")
                    def gsl(m):
                        return (gr if m < 2 else gz)[
                            :, (m % 2) * CH : (m % 2 + 1) * CH]

                    for m in range(4):
                        ms = slice(m * 128, (m + 1) * 128)
                        nc.tensor.matmul(gsl(m), wT0[:, ms], hT0[:, sl],
                                         start=True, stop=False)
                        nc.tensor.matmul(gsl(m), wT1[:, ms], hT1[:, sl],
                                         start=False, stop=False)
                    # rank-1 [x;1] fold: K=2 matmuls packed into distinct
                    # 32-row groups of the PE array -> run concurrently
                    for m in range(4):
                        mov = (xin[0:2, sl] if m == 0
                               else xob[32 * m : 32 * m + 2, sl])
                        nc.tensor.matmul(gsl(m),
                                         s3x2[32 * m : 32 * m + 2,
                                              m * 128 : (m + 1) * 128],
                                         mov,
                                         start=False, stop=True,
                                         tile_position=(32 * m, 0))
                    nc.scalar.activation(rz[:, 0 : 2 * CH], gr[:], AF.Sigmoid)
                    nc.scalar.activation(rz[:, 2 * CH : 4 * CH],
                                         gz[:], AF.Sigmoid)
                    gns = []
                    for tl in range(2):
                        gn = gnpool.tile([128, CH], f32, tag="gn", name="gn")
                        ms = slice((4 + tl) * 128, (5 + tl) * 128)
                        nc.tensor.matmul(gn[:], wT0[:, ms], hT0[:, sl],
                                         start=True, stop=False)
                        nc.tensor.matmul(gn[:], wT1[:, ms], hT1[:, sl],
                                         start=False, stop=True)
                        gns.append(gn)
                    # previous chunk's pred now that PE has fresh work queued
                    if pending is not None:
                        emit_tail(pending)
                    ta2 = wkpool.tile([128, 2 * CH], bf16, tag="ta2", name="ta2")
                    nn2 = wkpool.tile([128, 2 * CH], bf16, tag="nn2", name="nn2")
                    for tl, (hT, uc, gi) in enumerate(
                            ((hT0, uc0, gi0), (hT1, uc1, gi1))):
                        gn = gns[tl]
                        rr = rz[:, tl * CH : (tl + 1) * CH]
                        # rh = (gh_n + b_hh_n) * r
                        rh = wkpool.tile([128, CH], bf16, tag="rh", name="rh")
                        nc.vector.scalar_tensor_tensor(
                            rh[:], gn[:], uc[:, 2:3], rr, OP.add, OP.mult)
                        # ta = gi_n + rh
                        nc.vector.tensor_tensor(
                            ta2[:, tl * CH : (tl + 1) * CH], gi[:, sl], rh[:],
                            OP.add)
                    # n = tanh(ta) for both hidden tiles in one LUT pass
                    nc.scalar.activation(nn2[:], ta2[:], AF.Tanh)
                    for tl, hT in enumerate((hT0, hT1)):
                        nn = nn2[:, tl * CH : (tl + 1) * CH]
                        zz = rz[:, (2 + tl) * CH : (3 + tl) * CH]
                        hmn = wkpool.tile([128, CH], bf16, tag="hmn", name="hmn")
                        nc.gpsimd.tensor_tensor(hmn[:], hT[:, sl], nn, OP.subtract)
                        zh = wkpool.tile([128, CH], bf16, tag="zh", name="zh")
                        nc.gpsimd.tensor_tensor(zh[:], zz, hmn[:], OP.mult)
                        nc.gpsimd.tensor_tensor(hT[:, sl], nn, zh[:], OP.add)
                    pending = (t, c, xout)
            emit_tail(pending)

    nc.compile()
    return nc


def _prep_maps(encoder_out, w_proj, b_proj, W_ih, b_ih, W_hh, b_hh, w_out, b_out):
    f = np.float32
    u = (W_ih @ w_proj).astype(f)                    # [768]
    cvec = (W_ih @ b_proj + b_ih).astype(f)          # [768]
    s3 = np.zeros((2, H3), f)
    s3[0, : 2 * H] = u[: 2 * H]
    s3[1, : 2 * H] = cvec[: 2 * H] + b_hh[: 2 * H]
    s3[1, 2 * H :] = b_hh[2 * H :]
    s3x2 = np.zeros((128, H3), f)
    for m in range(4):
        s3x2[32 * m : 32 * m + 2, :] = s3
    uc = np.zeros((2, 128, 3), f)
    uc[0, :, 0] = u[2 * H : 2 * H + 128]
    uc[0, :, 1] = cvec[2 * H : 2 * H + 128]
    uc[0, :, 2] = b_hh[2 * H : 2 * H + 128]
    uc[1, :, 0] = u[2 * H + 128 :]
    uc[1, :, 1] = cvec[2 * H + 128 :]
    uc[1, :, 2] = b_hh[2 * H + 128 :]
    import ml_dtypes

    bf = ml_dtypes.bfloat16
    wo = np.ascontiguousarray(w_out.astype(f).reshape(2, 128, 1)).astype(bf)
    bo = b_out.astype(f).reshape(1, 1)
    whhT = np.ascontiguousarray(W_hh.astype(f).T).astype(bf)  # [256, 768]
    s3x2 = s3x2.astype(bf)
    maps = []
    for i in range(NCORES):
        h0T = np.ascontiguousarray(
            encoder_out[i * R : (i + 1) * R].astype(f).T
        ).astype(bf)
        maps.append(
            dict(h0T=h0T, whhT=whhT, s3x2=s3x2, uc=uc, wo=wo, bo=bo)
        )
    return maps


def _run(inputs, trace=False, **kw):
    import time

    from concourse.bass_utils import run_bass_kernel_spmd

    if "nc" not in _CACHE:
        _CACHE["nc"] = _build()
    nc = _CACHE["nc"]
    in_maps = _prep_maps(**inputs)
    res = None
    for attempt, pause in enumerate((0, 30, 120)):
        if pause:
            time.sleep(pause)  # transient NRT/axon device errors self-recover
        try:
            res = run_bass_kernel_spmd(nc, in_maps, core_ids=list(range(NCORES)),
                                       trace=trace, **kw)
            break
        except Exception:
            if attempt == 2:
                raise
    full = np.empty((N, T), np.float32)
    for i in range(NCORES):
        o = np.asarray(res.results[i]["out"]).astype(np.float32)
        full[i * R : (i + 1) * R] = o.T
    return full, res


def kernel(**inputs):
    inputs = {k: np.asarray(v) for k, v in inputs.items()}
    full, _ = _run(inputs)
    return full
